# revision 1
# baseline (speedup 1.0000x reference)
"""AttentionRefine kernel for Trainium2 (Bass/Tile), data-parallel over batch.

Reference computation (per batch b):
    f1 = W1 @ feat[b]          # [MID, N]   (1x1 conv as channel GEMM)
    f2 = W2 @ feat[b]          # [MID, N]
    s  = f1.T @ f2             # [N, N]
    A  = softmax(s, axis=-1)
    R  = A @ cam[b].T          # [N, C]
    out[b] = alpha * R.T + cam[b]

Kernel layout strategy (per core, 4 batches):
  - proj:  f1/f2 computed as [m(part), n] with contraction over c on partitions,
           using pre-transposed weights W^T [c, m].
  - sT:    computed directly in transposed [j(part), i] layout (swap f1/f2 roles),
           with the per-row -max folded in as a rank-1 matmul augmentation
           (extra K=1 row: ones x (-m_i)), so softmax needs no cross-partition work.
  - max:   an auxiliary low-precision s pass in [i(part), j] layout gives row maxes.
  - softmax: exp on ACT straight out of PSUM; row sums d_i via a ones-matmul
           (column sums of E^T); alpha/d_i broadcast back via a K=1 matmul.
  - final: out^T[c, i] = cam[b] @ E^T (unnorm.; cam^T via PE transpose); the
           alpha/d_i scaling + residual add fuse into the final PSUM evict.

8 cores, batch-sharded (4 each). No collectives.
"""

import numpy as np

import concourse.bacc as bacc
import concourse.mybir as mybir
import concourse.tile as tile
from concourse.bass_utils import run_bass_kernel_spmd
from concourse.masks import make_identity

F32 = mybir.dt.float32
F32R = mybir.dt.float32r
AF = mybir.ActivationFunctionType
ALU = mybir.AluOpType

# dtype knobs: F32R = TF32-like fast matmul, F32 = exact but 4x slower
DT_QK = F32R   # projections (f1/f2) and the s^T logits matmul
DT_PV = F32R   # E^T_norm and cam^T operands of the final matmul
TR_F32R = False  # PE transposes in f32r — rejected by BIR verifier, keep off
PROJ_EVICT_DVE = False

B_FULL = 32
N_CORES = 8
B_PER = B_FULL // N_CORES
C = 2048
KC = C // 128          # 16 channel chunks
MID = 256
N = 576                # 24*24 spatial
NH = N // 2            # 288 half, one PSUM bank per matmul target
JCH = [(0, 128), (128, 128), (256, 128), (384, 128), (512, 64)]  # N chunks


def hview(t, cols=None):
    """View a [P, 1024] PSUM tile as [P, 2, 288] (halves at bank offsets 0/512)."""
    v = t.rearrange("p (a b) -> p a b", a=2)
    return v[:, :, 0:NH]


def sview(ap):
    """View a [P, 576] contiguous slice as [P, 2, 288]."""
    return ap.rearrange("p (a b) -> p a b", a=2)


def build_nc(n_batches=B_PER, dt_qk=None, dt_pv=None, n_reps=1):
    dt_qk = DT_QK if dt_qk is None else dt_qk
    dt_pv = DT_PV if dt_pv is None else dt_pv

    nc = bacc.Bacc("TRN2", target_bir_lowering=False, debug=False,
                   num_devices=N_CORES)
    feat_d = nc.dram_tensor("feat", [n_batches, C, N], F32, kind="ExternalInput")
    cam_d = nc.dram_tensor("cam", [n_batches, C, N], F32, kind="ExternalInput")
    w1_d = nc.dram_tensor("w1", [MID, C], F32, kind="ExternalInput")
    w2_d = nc.dram_tensor("w2", [MID, C], F32, kind="ExternalInput")
    alpha_d = nc.dram_tensor("alpha", [1, 1], F32, kind="ExternalInput")
    out_d = nc.dram_tensor("out", [n_batches, C, N], F32, kind="ExternalOutput")

    with tile.TileContext(nc) as tc:
        with (
            tc.tile_pool(name="const", bufs=1) as pc,
            tc.tile_pool(name="wstage", bufs=4) as pws,
            tc.tile_pool(name="fstage", bufs=4) as pfs,
            tc.tile_pool(name="featr", bufs=1) as pfeat,
            tc.tile_pool(name="camp", bufs=1) as pcam,
            tc.tile_pool(name="camtp", bufs=1) as pcamt,
            tc.tile_pool(name="fsp", bufs=1) as pf,
            tc.tile_pool(name="etp", bufs=1) as pet,
            tc.tile_pool(name="outs", bufs=5) as pout,
            tc.tile_pool(name="pmm", bufs=7, space="PSUM") as pmm,
            tc.tile_pool(name="ptr", bufs=1, space="PSUM") as ptr,
        ):
            # ---- constants ----
            identity = pc.tile([128, 128], F32, name="identity")
            make_identity(nc, identity)
            onesr_qk = pc.tile([1, 128], dt_qk, name="onesr_qk")
            ones_row_f = pc.tile([1, 128], F32, name="ones_row_f")
            nc.gpsimd.memset(ones_row_f, 1.0)
            nc.gpsimd.tensor_copy(onesr_qk, ones_row_f)
            onesr_pv = pc.tile([1, 128], dt_pv, name="onesr_pv")
            nc.gpsimd.tensor_copy(onesr_pv, ones_row_f)
            ones_col_f = pc.tile([128, 1], F32, name="ones_col_f")
            nc.gpsimd.memset(ones_col_f, 1.0)
            onesc_pv = pc.tile([128, 1], dt_pv, name="onesc_pv")
            nc.gpsimd.tensor_copy(onesc_pv, ones_col_f)

            alpha_s = pc.tile([1, 1], F32, name="alpha_s")
            nc.sync.dma_start(out=alpha_s, in_=alpha_d.ap())

            if TR_F32R:
                identity_r = pc.tile([128, 128], F32R, name="identity_r")
                nc.gpsimd.tensor_copy(identity_r, identity)

            rm_col = pc.tile([128, 8], F32, name="rm_col")
            nc.gpsimd.memset(rm_col, 0.0)
            rm2 = pc.tile([128, 16], F32, name="rm2")
            snm = pc.tile([8, 128], F32, name="snm")
            negm_row = pc.tile([1, 640], F32, name="negm_row")
            negm_r = pc.tile([1, N], dt_qk, name="negm_r")
            d_s = pc.tile([1, N], F32, name="d_s")
            r_s = pc.tile([1, N], F32, name="r_s")
            r2_r = pc.tile([1, N], dt_pv, name="r2_r")

            # ---- weights: load + transpose to [c(part), m] ----
            w1t = pc.tile([128, KC * MID], dt_qk, name="w1t")
            w2t = pc.tile([128, KC * MID], dt_qk, name="w2t")
            for w_src, w_dst in ((w1_d, w1t), (w2_d, w2t)):
                for mc in range(2):
                    for kc4 in range(4):  # groups of 4 kc chunks
                        pt = ptr.tile([128, 512], F32, name="ptw", tag="ptw")
                        for q in range(4):
                            kc = kc4 * 4 + q
                            ws = pws.tile([128, 128], F32, name="ws", tag="ws")
                            nc.sync.dma_start(
                                out=ws,
                                in_=w_src.ap()[mc * 128:(mc + 1) * 128,
                                               kc * 128:(kc + 1) * 128])
                            nc.tensor.transpose(
                                pt[:, q * 128:(q + 1) * 128], ws, identity)
                        # evict 4 transposed blocks at once:
                        # dst columns kc*MID + mc*128, stride MID per kc
                        dst3 = w_dst.rearrange("p (k m) -> p k m", k=KC)[
                            :, kc4 * 4:kc4 * 4 + 4, mc * 128:(mc + 1) * 128]
                        src3 = pt.rearrange("p (a b) -> p a b", a=4)
                        nc.vector.tensor_copy(dst3, src3)

            # ---- main batch loop ----
            for b_iter in range(n_batches * n_reps):
                b = b_iter % n_batches
                # feat load (+ cast to dt_qk if needed)
                featr = pfeat.tile([128, KC * N], dt_qk, name="featr", tag="featr")
                for kc in range(KC):
                    src = feat_d.ap()[b, kc * 128:(kc + 1) * 128, :]
                    if dt_qk == F32:
                        nc.sync.dma_start(
                            out=featr[:, kc * N:(kc + 1) * N], in_=src)
                    else:
                        fs = pfs.tile([128, N], F32, name="fstage", tag="fstage")
                        nc.sync.dma_start(out=fs, in_=src)
                        nc.gpsimd.tensor_copy(
                            featr[:, kc * N:(kc + 1) * N], fs)

                # cam load (natural [c, j] layout, f32: transpose src + residual)
                cam_nat = pcam.tile([128, KC * N], F32, name="cam_nat",
                                    tag="cam_nat")
                for kc in range(KC):
                    nc.sync.dma_start(
                        out=cam_nat[:, kc * N:(kc + 1) * N],
                        in_=cam_d.ap()[b, kc * 128:(kc + 1) * 128, :])

                # ---- projections: f[i]s = W_i^T-contraction, [m(part), n] ----
                f1s = pf.tile([128, 2 * N], dt_qk, name="f1s", tag="f1s")
                f2s = pf.tile([128, 2 * N], dt_qk, name="f2s", tag="f2s")
                for w_t, f_dst in ((w1t, f1s), (w2t, f2s)):
                    for mc in range(2):
                        for h in range(2):
                            pp = pmm.tile([128, NH], F32, name="ppr", tag="ppr")
                            for kc in range(KC):
                                nc.tensor.matmul(
                                    pp,
                                    lhsT=w_t[:, kc * MID + mc * 128:
                                             kc * MID + (mc + 1) * 128],
                                    rhs=featr[:, kc * N + h * NH:
                                              kc * N + (h + 1) * NH],
                                    start=(kc == 0), stop=(kc == KC - 1))
                            if PROJ_EVICT_DVE:
                                nc.vector.tensor_copy(
                                    f_dst[:, mc * N + h * NH:
                                          mc * N + (h + 1) * NH], pp)
                            else:
                                nc.scalar.copy(
                                    f_dst[:, mc * N + h * NH:
                                          mc * N + (h + 1) * NH], pp)

                # ---- aux pass: s[i,j] row maxes (always fast dtype ok) ----
                for ic, (i0, isz) in enumerate(JCH):
                    for h in range(2):
                        ps = pmm.tile([128, NH], F32, name="psmax", tag="ppr")
                        for mc in range(2):
                            nc.tensor.matmul(
                                ps[0:isz, :],
                                lhsT=f1s[:, mc * N + i0:mc * N + i0 + isz],
                                rhs=f2s[:, mc * N + h * NH:mc * N + (h + 1) * NH],
                                start=(mc == 0), stop=(mc == 1))
                        nc.vector.reduce_max(
                            rm2[0:isz, 2 * ic + h:2 * ic + h + 1], ps[0:isz, :],
                            axis=mybir.AxisListType.X)
                    nc.vector.reduce_max(
                        rm_col[0:isz, ic:ic + 1], rm2[0:isz, 2 * ic:2 * ic + 2],
                        axis=mybir.AxisListType.X, negate=True)

                # ---- cam^T via PE transposes (fills PE while DVE reduces) ----
                camt = pcamt.tile([128, 5 * C], dt_pv, name="camt", tag="camt")
                for jc, (j0, jsz) in enumerate(JCH):
                    for cc4 in range(4):  # 4 groups of 4 c-chunks
                        pt = pmm.tile([128, 512], F32, name="ptc", tag="ppr")
                        for q in range(4):
                            cc = cc4 * 4 + q
                            src = cam_nat[:, cc * N + j0:cc * N + j0 + jsz]
                            nc.tensor.transpose(
                                pt[0:jsz, q * 128:(q + 1) * 128],
                                src, identity)
                        # ACT is idle here; keep DVE for the softmax path
                        nc.scalar.copy(
                            camt[0:jsz, jc * C + cc4 * 512:
                                 jc * C + (cc4 + 1) * 512],
                            pt[0:jsz, :])

                # ---- negm_row: [1, N] = -rowmax, via PE transpose of rm_col ----
                pnm = ptr.tile([128, 512], F32, name="pnm", tag="ptw")
                nc.tensor.transpose(pnm[0:8, 0:128], rm_col, identity)
                nc.vector.tensor_copy(snm, pnm[0:8, 0:128])
                # consolidate [5, 128] partition rows into one [1, 640] row
                nc.sync.dma_start(
                    out=negm_row.rearrange("a (b c) -> a b c", b=5),
                    in_=snm[0:5, :])
                if dt_qk == F32:
                    negm_use = negm_row[0:1, 0:N]
                else:
                    nc.gpsimd.tensor_copy(negm_r, negm_row[0:1, 0:N])
                    negm_use = negm_r

                # ---- s^T (+ fused -max) and exp -> E^T ----
                et = pet.tile([128, 5 * N], dt_pv, name="et", tag="et")
                for jc, (j0, jsz) in enumerate(JCH):
                    for h in range(2):
                        ps = pmm.tile([128, NH], F32, name="pst", tag="ppr")
                        for mc in range(2):
                            nc.tensor.matmul(
                                ps[0:jsz, :],
                                lhsT=f2s[:, mc * N + j0:mc * N + j0 + jsz],
                                rhs=f1s[:, mc * N + h * NH:mc * N + (h + 1) * NH],
                                start=(mc == 0), stop=False)
                        nc.tensor.matmul(
                            ps[0:jsz, :],
                            lhsT=onesr_qk[0:1, 0:jsz],
                            rhs=negm_use[0:1, h * NH:(h + 1) * NH],
                            start=False, stop=True)
                        nc.scalar.activation(
                            et[0:jsz, jc * N + h * NH:jc * N + (h + 1) * NH],
                            ps[0:jsz, :], AF.Exp)

                # ---- d = column sums of E^T; r2 = alpha / d ----
                for h in range(2):
                    pd = pmm.tile([128, NH], F32, name="pd", tag="ppr")
                    for jc, (j0, jsz) in enumerate(JCH):
                        nc.tensor.matmul(
                            pd[0:1, :],
                            lhsT=onesc_pv[0:jsz, 0:1],
                            rhs=et[0:jsz, jc * N + h * NH:jc * N + (h + 1) * NH],
                            start=(jc == 0), stop=(jc == 4))
                    nc.vector.tensor_copy(d_s[0:1, h * NH:(h + 1) * NH], pd[0:1, :])
                nc.vector.reciprocal(r_s, d_s)
                nc.vector.tensor_scalar_mul(r_s, r_s, alpha_s[0:1, 0:1])
                nc.gpsimd.tensor_copy(r2_r, r_s)

                # ---- broadcast r2 = alpha/d to all partitions (SBUF copy) ----
                rbc_s = pc.tile([128, N], F32, name="rbc_s")
                for h in range(2):
                    prb = pmm.tile([128, NH], F32, name="prb", tag="ppr")
                    nc.tensor.matmul(
                        prb,
                        lhsT=onesr_pv[0:1, 0:128],
                        rhs=r2_r[0:1, h * NH:(h + 1) * NH],
                        start=True, stop=True)
                    nc.vector.tensor_copy(rbc_s[:, h * NH:(h + 1) * NH], prb)

                # ---- final: out^T[c, i] = cam @ E^T (unnormalized);
                #      evict applies alpha/d_i scaling + residual ----
                for cc in range(KC):
                    o_s = pout.tile([128, N], F32, name="o_s", tag="o_s")
                    for h in range(2):
                        po = pmm.tile([128, NH], F32, name="po", tag="ppr")
                        for jc, (j0, jsz) in enumerate(JCH):
                            nc.tensor.matmul(
                                po,
                                lhsT=camt[0:jsz, jc * C + cc * 128:
                                          jc * C + (cc + 1) * 128],
                                rhs=et[0:jsz, jc * N + h * NH:
                                       jc * N + (h + 1) * NH],
                                start=(jc == 0), stop=(jc == 4))
                        nc.vector.tensor_tensor(
                            o_s[:, h * NH:(h + 1) * NH], po,
                            rbc_s[:, h * NH:(h + 1) * NH], op=ALU.mult)
                    eng = nc.vector if cc % 2 == 0 else nc.gpsimd
                    eng.tensor_tensor(
                        o_s, o_s, cam_nat[:, cc * N:(cc + 1) * N], op=ALU.add)
                    nc.sync.dma_start(
                        out=out_d.ap()[b, cc * 128:(cc + 1) * 128, :], in_=o_s)

    nc.compile()
    return nc


_NC_CACHE = {}


def _get_nc():
    key = (DT_QK, DT_PV, B_PER)
    if key not in _NC_CACHE:
        _NC_CACHE[key] = build_nc(B_PER)
    return _NC_CACHE[key]


def make_in_maps(cam, feat, W1, W2, alpha):
    cam = np.ascontiguousarray(np.asarray(cam, np.float32).reshape(B_FULL, C, N))
    feat = np.ascontiguousarray(np.asarray(feat, np.float32).reshape(B_FULL, C, N))
    W1 = np.ascontiguousarray(np.asarray(W1, np.float32))
    W2 = np.ascontiguousarray(np.asarray(W2, np.float32))
    alpha = np.asarray(alpha, np.float32).reshape(1, 1)
    return [
        {"feat": feat[i * B_PER:(i + 1) * B_PER],
         "cam": cam[i * B_PER:(i + 1) * B_PER],
         "w1": W1, "w2": W2, "alpha": alpha}
        for i in range(N_CORES)
    ]


def kernel(cam, feat, W1, W2, alpha):
    H = W = 24
    nc = _get_nc()
    in_maps = make_in_maps(cam, feat, W1, W2, alpha)
    res = run_bass_kernel_spmd(nc, in_maps, list(range(N_CORES)))
    out = np.concatenate([res.results[i]["out"] for i in range(N_CORES)], axis=0)
    return out.reshape(B_FULL, C, H, W).astype(np.float32)

